# revision 57
# baseline (speedup 1.0000x reference)
"""Trainium2 Bass kernel for pre-norm causal attention block.

Module: out = x + Wo(attn(LN(x))) with fused QKV, 16 heads, causal mask.
Shapes (hardcoded): x [2, 2048, 1024], wqkv [1024, 3072], wo [1024, 1024].

Sharding (8 cores, one program SPMD):
  core c = 4*b + s handles batch b, global heads [4s, 4s+4).  The attention
  context is exchanged with 4 small per-qt AllToAlls (64-token sub-chunks:
  core r owns tokens {512*qt + 64*r + i}), each overlapped with the next
  qt's attention compute; the receiver side does the output projection per
  chunk as it lands.

Per-core dataflow (feature-on-partitions, transposed):
  1. LN stats via ones-matmul on PE; fast-rsqrt NR on DVE.  LN mean
     correction is folded into the projections as an extra rank-1/2 matmul
     (lhsT = [-C; b2], rhs = [mu; 1]); LN scale r is applied as one
     tensor mult on Q, folded into the exp scale (r_k/8, per-partition AP)
     on K, and one tensor_scalar on V.  K/V biases are exact-folded
     (K bias cancels in softmax; V bias folded into bo on host).
  2. Scores per head-pair into one [128, 1024] PSUM tile, single exp per
     pair (split + masked on diagonal tiles), ctx accumulated per head in
     [65, 512] PSUM (row 64 = softmax denominator).
  3. Sender-side normalize: reciprocal_approx_fast on the 4 den rows,
     PE broadcast, one DVE mult -> normalized bf16 ctx^T; staged and
     shipped via the per-qt AllToAll (Shared output buffers).
  4. Receiver (interleaved per call): gather 8x[128,64] ctx blocks per
     batch in one DMA, output projection + residual + bias, store.
"""

import sys

for _p in ("/opt/trn_rl_repo",):
    if _p not in sys.path:
        sys.path.insert(0, _p)

import ml_dtypes
import numpy as np

import concourse.bass as bass
import concourse.mybir as mybir
import concourse.tile as tile
from concourse import bacc
from concourse.bass_utils import run_bass_kernel_spmd

F32 = mybir.dt.float32
F32R = mybir.dt.float32r
BF16 = mybir.dt.bfloat16
I32 = mybir.dt.int32
AF = mybir.ActivationFunctionType
ALU = mybir.AluOpType

N_CORES = 8
B, S, H, D = 2, 2048, 16, 64
DIM = H * D              # 1024
HL = 4                   # heads per core
DL = HL * D              # 256 local head features
WC = 64                  # per-call sub-chunk width (tokens)
EPS = 1e-6
KT = 128                 # k-tile (partition) width
NT = 512                 # matmul free-dim tile
FT = DIM // KT           # 8 feature tiles
ST = S // KT             # 16 seq tiles of 128
QT = S // NT             # 4 q-tiles of 512

_CACHE = {}


def _build(with_qbias):
    nc = bacc.Bacc("TRN2", target_bir_lowering=False, debug=False,
                   num_devices=N_CORES)

    # ---- I/O ----
    xbf_d = nc.dram_tensor("xbf", [DIM, S], BF16, kind="ExternalInput")
    xres_d = nc.dram_tensor("xres", [DIM, 2 * WC * QT], F32,
                            kind="ExternalInput")
    wqk_d = nc.dram_tensor("wqk", [DIM, 2 * DL], BF16, kind="ExternalInput")
    wv_d = nc.dram_tensor("wv", [DIM, DL], BF16, kind="ExternalInput")
    wo_d = nc.dram_tensor("wo", [DIM, DIM], BF16, kind="ExternalInput")
    augq_d = nc.dram_tensor("augq", [2, 2 * DL], BF16, kind="ExternalInput")
    ncv_d = nc.dram_tensor("ncv", [1, DL], BF16, kind="ExternalInput")
    bq_d = nc.dram_tensor("bq", [128, 2], F32, kind="ExternalInput")
    sel_d = nc.dram_tensor("sel", [2, 128], BF16, kind="ExternalInput")
    tri_d = nc.dram_tensor("tri", [128, 128], BF16, kind="ExternalInput")
    bo_d = nc.dram_tensor("bo_col", [128, FT], F32, kind="ExternalInput")
    y_d = nc.dram_tensor("y", [DIM, 2 * WC * QT], F32, kind="ExternalOutput")

    # ---- DRAM scratch ----
    stats_dram = nc.dram_tensor("stats_dram", [2, S], F32)
    a2a_in = [nc.dram_tensor(f"a2a_in{t}", [N_CORES, DL, WC], BF16)
              for t in range(QT)]
    a2a_out = [nc.dram_tensor(f"a2a_out{t}", [N_CORES, DL, WC], BF16)
               for t in range(QT)]

    with tile.TileContext(nc) as tc:
        import contextlib
        with contextlib.ExitStack() as ctx:
            _build_body(ctx, tc, nc, locals(), with_qbias)
    nc.compile()
    return nc


def _build_body(ctx, tc, nc, t, with_qbias):
    import math
    xbf_d, xres_d, wqk_d, wv_d, wo_d = (t["xbf_d"], t["xres_d"], t["wqk_d"],
                                        t["wv_d"], t["wo_d"])
    augq_d, ncv_d, bq_d, tri_d, bo_d, y_d = (
        t["augq_d"], t["ncv_d"], t["bq_d"], t["tri_d"], t["bo_d"], t["y_d"])
    sel_d = t["sel_d"]
    stats_dram, a2a_in, a2a_out = t["stats_dram"], t["a2a_in"], t["a2a_out"]

    P = 128
    sing = ctx.enter_context(tc.tile_pool(name="sing", bufs=1))
    # persistent SBUF tiles
    xbf = [sing.tile([P, S], BF16, tag=f"xbf{i}", name=f"xbf{i}")
           for i in range(FT)]
    xres = [sing.tile([P, 2 * WC * QT], F32, tag=f"xres{i}", name=f"xres{i}")
            for i in range(FT)]
    wqk = [sing.tile([P, 2 * DL], BF16, tag=f"wqk{i}", name=f"wqk{i}")
           for i in range(FT)]
    wv = [sing.tile([P, DL], BF16, tag=f"wv{i}", name=f"wv{i}")
          for i in range(FT)]
    wo = [sing.tile([P, DIM], BF16, tag=f"wo{i}", name=f"wo{i}")
          for i in range(FT)]
    qkT = [sing.tile([P, S], BF16, tag=f"qkT{i}", name=f"qkT{i}")
           for i in range(4)]
    vaug = [sing.tile([P, HL * (D + 1)], BF16, tag=f"vaug{i}",
                      name=f"vaug{i}") for i in range(ST)]
    rB = [sing.tile([P, NT], F32, tag=f"rB{i}", name=f"rB{i}")
          for i in range(QT)]
    augq = sing.tile([2, 2 * DL], BF16, tag="augq")
    ncv = sing.tile([1, DL], BF16, tag="ncv")
    bq_c = sing.tile([P, 2], F32, tag="bq")
    sel_a = sing.tile([1, P], BF16, tag="sel_a")
    sel_b = sing.tile([1, P], BF16, tag="sel_b")
    tri = sing.tile([P, P], BF16, tag="tri")
    bo_c = sing.tile([P, FT], F32, tag="bo")
    ones = sing.tile([P, 1], BF16, tag="ones")
    ones1 = sing.tile([1, P], BF16, tag="ones1")
    mu2 = sing.tile([2, S], BF16, tag="mu2")
    r_row = sing.tile([1, S], BF16, tag="r_row")
    sgP = sing.tile([P, ST], F32, tag="sgP")
    rcP = sing.tile([P, ST], F32, tag="rcP")
    idn = sing.tile([P, P], F32, tag="idn")

    # input DMAs -- xbf first (stats critical path), weights next, rest last
    for i in range(FT):
        nc.sync.dma_start(out=xbf[i], in_=xbf_d[i * P:(i + 1) * P, :])
    for i in range(FT):
        nc.sync.dma_start(out=wqk[i], in_=wqk_d[i * P:(i + 1) * P, :])
    for i in range(FT):
        nc.sync.dma_start(out=wv[i], in_=wv_d[i * P:(i + 1) * P, :])
    nc.sync.dma_start(out=augq, in_=augq_d[:])
    nc.sync.dma_start(out=ncv, in_=ncv_d[:])
    nc.sync.dma_start(out=bq_c, in_=bq_d[:])
    nc.sync.dma_start(out=sel_a, in_=sel_d[0:1, :])
    nc.sync.dma_start(out=sel_b, in_=sel_d[1:2, :])
    nc.sync.dma_start(out=tri, in_=tri_d[:])
    nc.sync.dma_start(out=bo_c, in_=bo_d[:])
    for i in range(FT):
        nc.sync.dma_start(out=wo[i], in_=wo_d[i * P:(i + 1) * P, :])
        nc.sync.dma_start(out=xres[i], in_=xres_d[i * P:(i + 1) * P, :])
    nc.vector.memset(ones, 1.0)
    nc.vector.memset(ones1, 1.0)
    nc.vector.memset(mu2, 1.0)       # row 0 overwritten by mu DMA below
    from concourse.masks import make_identity
    make_identity(nc, idn)

    # ---- 1. LN stats: column sums of x and x^2 via ones-matmul ----
    with tc.tile_pool(name="ps_st", bufs=4, space="PSUM") as ps_st, \
         tc.tile_pool(name="sqp", bufs=2) as sqp:
        stats_sa = sqp.tile([1, S], F32, tag="stats_sa", bufs=1)
        stats_sq = sqp.tile([1, S], F32, tag="stats_sq", bufs=1)
        sps = [ps_st.tile([1, NT], F32, tag="sum", name=f"sum{nt}")
               for nt in range(QT)]
        qps = [ps_st.tile([1, NT], F32, tag="sq", name=f"sqp{nt}")
               for nt in range(QT)]
        for k in range(FT):
            sq = sqp.tile([P, S], BF16, tag="sq", name="sq")
            nc.vector.tensor_mul(sq, xbf[k], xbf[k])
            for nt in range(QT):
                sl = slice(nt * NT, (nt + 1) * NT)
                nc.tensor.matmul(sps[nt], ones, xbf[k][:, sl],
                                 start=(k == 0), stop=(k == FT - 1))
                nc.tensor.matmul(qps[nt], ones, sq[:, sl],
                                 start=(k == 0), stop=(k == FT - 1))
        for nt in range(QT):
            sl = slice(nt * NT, (nt + 1) * NT)
            nc.vector.tensor_copy(stats_sa[:, sl], sps[nt])
            nc.vector.tensor_copy(stats_sq[:, sl], qps[nt])
    # QK main matmuls for mt=0 hoisted here: keeps PE busy during the
    # stats DRAM bounce + rsqrt chain (their aug/epilogue comes later).
    import contextlib as _ctl
    qk_ctx = _ctl.ExitStack()
    ps_qk = qk_ctx.enter_context(
        tc.tile_pool(name="ps_qk", bufs=5, space="PSUM"))
    tmp = qk_ctx.enter_context(tc.tile_pool(name="tmp", bufs=3))
    pre = []
    for nt in range(QT):
        ps = ps_qk.tile([P, NT], F32, tag="qk", name="qk")
        for k in range(FT):
            nc.tensor.matmul(ps, wqk[k][:, 0:P],
                             xbf[k][:, nt * NT:(nt + 1) * NT],
                             start=(k == 0), stop=False)
        pre.append(ps)
    nc.sync.dma_start(out=stats_dram[0:1], in_=stats_sa[:])
    nc.sync.dma_start(out=stats_dram[1:2], in_=stats_sq[:])
    # [16,128] reads, math at 16 partitions, then flatten (SBUF->SBUF DMA)
    sPT = sing.tile([16, P], F32, tag="sPT")
    qPT = sing.tile([16, P], F32, tag="qPT")
    nc.sync.dma_start(out=sPT, in_=stats_dram[0].rearrange("(j p) -> j p",
                                                           j=16))
    nc.sync.dma_start(out=qPT, in_=stats_dram[1].rearrange("(j p) -> j p",
                                                           j=16))
    muT = sing.tile([16, P], F32, tag="muT")
    nc.vector.tensor_scalar(muT, sPT, 1.0 / DIM, None, op0=ALU.mult)
    nc.vector.tensor_scalar(qPT, qPT, 1.0 / DIM, None, op0=ALU.mult)
    t0 = sing.tile([16, P], F32, tag="t0")
    nc.vector.tensor_mul(t0, muT, muT)
    nc.vector.tensor_sub(t0, qPT, t0)
    nc.vector.tensor_scalar(t0, t0, EPS, None, op0=ALU.add)
    # rsqrt via fast-inverse-square-root seed + 3 Newton steps
    rT = sing.tile([16, P], F32, tag="rT")
    t1s = sing.tile([16, P], F32, tag="t1s")
    nc.vector.tensor_scalar(rT[:].bitcast(I32), t0[:].bitcast(I32), 1, None,
                            op0=ALU.logical_shift_right)
    nc.vector.tensor_scalar(rT[:].bitcast(I32), rT[:].bitcast(I32), -1, None,
                            op0=ALU.bitwise_xor)
    nc.vector.tensor_scalar(rT[:].bitcast(I32), rT[:].bitcast(I32),
                            0x5F3759E0, None, op0=ALU.add)
    for _ in range(3):
        nc.vector.tensor_mul(t1s, rT, rT)
        nc.vector.tensor_mul(t1s, t1s, t0)
        nc.vector.tensor_scalar(t1s, t1s, -0.5, 1.5, op0=ALU.mult,
                                op1=ALU.add)
        nc.vector.tensor_mul(rT, rT, t1s)
    muTb = sing.tile([16, P], BF16, tag="muTb")
    nc.vector.tensor_copy(muTb, muT)
    nc.sync.dma_start(out=mu2[0:1, :], in_=muTb[:])
    rTb = sing.tile([16, P], BF16, tag="rTb")
    nc.vector.tensor_copy(rTb, rT)
    nc.sync.dma_start(out=r_row, in_=rTb[:])
    with tc.tile_pool(name="ps_bc", bufs=1, space="PSUM") as ps_bc:
        for nt in range(QT):
            sl = slice(nt * NT, (nt + 1) * NT)
            bp = ps_bc.tile([P, NT], F32, tag="bc", name="bc")
            nc.tensor.matmul(bp, ones1, r_row[:, sl], start=True, stop=True)
            nc.vector.tensor_copy(rB[nt], bp)
        # column layout via PE transpose: rcP (V epilogue), sgP (exp scale)
        tp = ps_bc.tile([P, 16], F32, tag="tp", name="tp")
        nc.tensor.transpose(tp, rT[:], idn[0:16, 0:16])
        nc.vector.tensor_copy(rcP, tp)
        nc.vector.tensor_scalar(sgP, rcP, 1.0 / math.sqrt(D), None,
                                op0=ALU.mult)

    # ---- 2. QK projection ----
    if True:
        for mt in range(4):          # qkT M-tiles (Q01 Q23 K01 K23)
            for nt in range(QT):
                sl = slice(nt * NT, (nt + 1) * NT)
                if mt == 0:
                    ps = pre[nt]
                else:
                    ps = ps_qk.tile([P, NT], F32, tag="qk", name="qk")
                    for k in range(FT):
                        nc.tensor.matmul(
                            ps, wqk[k][:, mt * P:(mt + 1) * P],
                            xbf[k][:, sl], start=(k == 0), stop=False)
                nc.tensor.matmul(ps, augq[:, mt * P:(mt + 1) * P],
                                 mu2[:, sl], start=False, stop=True)
                if mt < 2:
                    # Q: apply LN scale r (per-token broadcast)
                    if with_qbias:
                        t1 = tmp.tile([P, NT], F32, tag="t1")
                        nc.vector.tensor_mul(t1, ps, rB[nt])
                        nc.vector.tensor_scalar(
                            qkT[mt][:, sl], t1, bq_c[:, mt:mt + 1], None,
                            op0=ALU.add)
                    else:
                        nc.vector.tensor_mul(qkT[mt][:, sl], ps, rB[nt])
                else:
                    # K: r folded into exp scale; plain copy to bf16
                    nc.scalar.copy(qkT[mt][:, sl], ps)

    qk_ctx.close()

    # ---- 3. V projection (all tiles up front) ----
    with tc.tile_pool(name="ps_v", bufs=2, space="PSUM") as ps_v:
        for st in range(ST):
            ps = ps_v.tile([P, DL], F32, tag="v", name="v")
            for k in range(FT):
                nc.tensor.matmul(
                    ps, xbf[k][:, st * P:(st + 1) * P], wv[k],
                    start=(k == 0), stop=False)
            nc.tensor.matmul(ps, mu2[0:1, st * P:(st + 1) * P], ncv,
                             start=False, stop=True)
            nc.vector.tensor_scalar(
                vaug[st][:].rearrange("p (h e) -> p h e", h=HL)[:, :, 0:D],
                ps.rearrange("p (h d) -> p h d", h=HL),
                rcP[:, st:st + 1], None, op0=ALU.mult)
            nc.vector.memset(
                vaug[st][:].rearrange("p (h e) -> p h e", h=HL)[:, :,
                                                                D:D + 1],
                1.0)

    # ---- 4. attention + per-qt A2A + interleaved receiver ----
    with tc.tile_pool(name="ps_sc", bufs=2, space="PSUM") as ps_sc, \
         tc.tile_pool(name="ps_cx", bufs=1, space="PSUM") as ps_cx, \
         tc.tile_pool(name="esp", bufs=4) as esp, \
         tc.tile_pool(name="ctxp", bufs=4) as ctxp, \
         tc.tile_pool(name="denp", bufs=2) as denp, \
         tc.tile_pool(name="cap", bufs=4) as cap, \
         tc.tile_pool(name="yp", bufs=4) as yp:

        def receiver(call):
            # gather ctx blocks: one DMA per batch
            ca = []
            for b2 in range(2):
                cat = cap.tile([P, FT, WC], BF16, tag="ca", name="ca")
                nc.sync.dma_start(
                    out=cat[:],
                    in_=a2a_out[call][4 * b2:4 * b2 + 4].rearrange(
                        "g (f p) q -> p (g f) q", f=2))
                ca.append(cat)
            for mt in range(FT):
                pof = ps_sc.tile([P, 2 * NT], F32, tag="sc", name="po")
                for k in range(FT):
                    for b2 in range(2):
                        nc.tensor.matmul(
                            pof[:, b2 * NT:b2 * NT + WC],
                            wo[k][:, mt * P:(mt + 1) * P],
                            ca[b2][:, k, :],
                            start=(k == 0), stop=(k == FT - 1))
                ysb = yp.tile([P, 2 * WC], F32, tag="ysb", name="ysb")
                yout = yp.tile([P, 2 * WC], F32, tag="yout", name="yout")
                csl = slice(call * 2 * WC, (call + 1) * 2 * WC)
                for b2 in range(2):
                    nc.vector.tensor_add(
                        ysb[:, b2 * WC:(b2 + 1) * WC],
                        pof[:, b2 * NT:b2 * NT + WC],
                        xres[mt][:, call * 2 * WC + b2 * WC:
                                 call * 2 * WC + (b2 + 1) * WC])
                nc.scalar.activation(yout, ysb, AF.Identity,
                                     bias=bo_c[:, mt:mt + 1])
                nc.sync.dma_start(out=y_d[mt * P:(mt + 1) * P, csl],
                                  in_=yout)

        for qt in range(QT):
            q0 = qt * NT
            cxs = [ps_cx.tile([D + 1, NT], F32, tag=f"cx{hl}",
                              name=f"cx{hl}") for hl in range(HL)]
            def emit_ctx(es_pair, kt):
                for pr in range(2):
                    for u in range(2):
                        hl = 2 * pr + u
                        nc.tensor.matmul(
                            cxs[hl],
                            vaug[kt][:, hl * (D + 1):(hl + 1) * (D + 1)],
                            es_pair[pr][:, u * NT:(u + 1) * NT],
                            start=(kt == 0), stop=(kt == 4 * qt + 3))

            pend = None            # (es_pair, kt) deferred by one k-tile
            for kt in range(4 * qt + 4):
                k0 = kt * KT
                dlt = k0 - q0          # >0 only on diagonal k-tiles
                cur = []
                for pr in range(2):    # head pairs (2pr, 2pr+1)
                    sc = ps_sc.tile([P, 2 * NT], F32, tag="sc", name="sc")
                    es = esp.tile([P, 2 * NT], BF16, tag="es", name="es")
                    for u in range(2):
                        hp = slice(D * u, D * u + D)
                        off = u * NT
                        if dlt > 0:
                            nc.vector.memset(es[:, off:off + dlt], 0.0)
                            nc.tensor.matmul(
                                sc[:, off + dlt:off + NT],
                                qkT[2 + pr][hp, k0:k0 + KT],
                                qkT[pr][hp, q0 + dlt:q0 + NT],
                                start=True, stop=True)
                        else:
                            nc.tensor.matmul(
                                sc[:, off:off + NT],
                                qkT[2 + pr][hp, k0:k0 + KT],
                                qkT[pr][hp, q0:q0 + NT],
                                start=True, stop=True)
                    if dlt > 0:
                        for u in range(2):
                            off = u * NT
                            nc.scalar.activation(
                                es[:, off + dlt:off + NT],
                                sc[:, off + dlt:off + NT], AF.Exp,
                                scale=sgP[:, kt:kt + 1])
                    else:
                        nc.scalar.activation(es, sc, AF.Exp,
                                             scale=sgP[:, kt:kt + 1])
                    if dlt >= 0 and kt >= 4 * qt:   # diagonal triangle
                        for u in range(2):
                            off = u * NT
                            nc.vector.tensor_mul(
                                es[:, off + dlt:off + dlt + KT],
                                es[:, off + dlt:off + dlt + KT], tri)
                    cur.append(es)
                if pend is not None:
                    emit_ctx(*pend)
                pend = (cur, kt)
            emit_ctx(*pend)
            # qt tail: sender-side normalize + stage + collective
            dens = denp.tile([1, HL * NT], BF16, tag="dens", name="dens")
            for hl in range(HL):
                nc.scalar.copy(dens[:, hl * NT:(hl + 1) * NT],
                               cxs[hl][D:D + 1, :])
            for pr in range(2):
                rbt = ps_sc.tile([P, 2 * NT], F32, tag="sc", name="rb")
                for u in range(2):
                    hl = 2 * pr + u
                    nc.tensor.matmul(
                        rbt[:, 0:NT],
                        sel_a if u == 0 else sel_b,
                        dens[:, hl * NT:(hl + 1) * NT],
                        start=(u == 0), stop=(u == 1))
                dsb = denp.tile([P, NT], F32, tag="dsb", name="dsb")
                nc.vector.tensor_copy(dsb, rbt[:, 0:NT])
                rcb = denp.tile([P, NT], F32, tag="rcb", name="rcb")
                nc.vector.reciprocal_approx_fast(out=rcb[:], in_=dsb[:])
                for u in range(2):
                    hl = 2 * pr + u
                    ct = ctxp.tile([D, NT], BF16, tag="ct", name="ct")
                    nc.vector.tensor_mul(ct, cxs[hl][0:D, :],
                                         rcb[u * D:(u + 1) * D, :])
                    nc.sync.dma_start(
                            out=a2a_in[qt][:, hl * D:(hl + 1) * D,
                                           :].rearrange("d p q -> p d q"),
                            in_=ct[:].rearrange("p (d q) -> p d q",
                                                d=N_CORES))
            nc.gpsimd.collective_compute(
                "AllToAll", ALU.bypass,
                replica_groups=[list(range(N_CORES))],
                ins=[a2a_in[qt][:].opt()], outs=[a2a_out[qt][:].opt()],
                unique_tensors="Yes")
            if qt >= 1:
                receiver(qt - 1)
        receiver(QT - 1)


def _prep_inputs(x, ln_g, ln_b, wqkv, bqkv, wo, bo):
    """Host-side sharding / folding. Returns per-core input dicts."""
    f32 = np.float32
    bf16 = ml_dtypes.bfloat16
    x = np.asarray(x, f32)
    wg = (np.asarray(wqkv, f32) * np.asarray(ln_g, f32)[:, None])
    tri = (np.arange(128)[None, :] >= np.arange(128)[:, None]).astype(bf16)
    wo_f = np.asarray(wo, f32)
    wo_bf = wo_f.astype(bf16)
    lnb = np.asarray(ln_b, f32)
    bq = np.asarray(bqkv, f32)
    bo_f = np.asarray(bo, f32)

    xT = [np.ascontiguousarray(x[b].T) for b in range(B)]
    xbf = [t.astype(bf16) for t in xT]

    # V bias folded through Wo: full ctx bias vector (all head groups)
    b2v_full = np.zeros(DIM, f32)
    for s in range(4):
        vs = slice(2 * DIM + DL * s, 2 * DIM + DL * s + DL)
        wv_f = wg[:, vs]
        b2v_full[DL * s:DL * s + DL] = bq[vs] + wv_f.T @ lnb
    bo2 = bo_f + b2v_full @ wo_f
    sel2 = np.zeros((2, 128), np.float32)
    sel2[0, 0:64] = 1.0
    sel2[1, 64:128] = 1.0
    sel2 = sel2.astype(bf16)

    maps = []
    qbias = False
    for c in range(N_CORES):
        b, s = divmod(c, 4)
        qs = slice(DL * s, DL * s + DL)
        ks = slice(DIM + DL * s, DIM + DL * s + DL)
        vs = slice(2 * DIM + DL * s, 2 * DIM + DL * s + DL)
        wqk_l = np.concatenate([wg[:, qs], wg[:, ks]], axis=1).astype(bf16)
        wv_l = wg[:, vs].astype(bf16)
        wqk_f = wqk_l.astype(f32)
        wv_f = wv_l.astype(f32)
        cqk = wqk_f.sum(0)                       # [512]
        b2q = bq[qs] + wqk_f[:, 0:DL].T @ lnb    # Q bias (post-scale ref!)
        cv = wv_f.sum(0)                         # [256]
        if np.abs(b2q).max() > 0:
            qbias = True
        augq = np.stack([-cqk, np.zeros(2 * DL, f32)]).astype(bf16)
        # tokens for core c: 512*qt + 64*c + i, cols ordered [qt][b2][64]
        toks = (512 * np.arange(QT)[:, None] + WC * c
                + np.arange(WC)[None, :]).reshape(-1)
        xres_c = np.stack([xT[b2][:, toks] for b2 in range(2)], axis=1)
        xres_c = xres_c.reshape(DIM, 2, QT, WC).transpose(0, 2, 1, 3)
        xres_c = np.ascontiguousarray(xres_c.reshape(DIM, 2 * WC * QT))
        maps.append({
            "xbf": xbf[b],
            "xres": xres_c,
            "wqk": wqk_l,
            "wv": wv_l,
            "wo": wo_bf,
            "augq": augq,
            "ncv": np.ascontiguousarray(-cv[None, :]).astype(bf16),
            "bq": np.ascontiguousarray(
                b2q.reshape(2, 128).T.astype(f32)),
            "tri": tri,
            "bo_col": np.ascontiguousarray(bo2.reshape(FT, 128).T),
            "sel": sel2,
        })
    maps_qbias = qbias
    return maps, maps_qbias


def kernel(**inputs):
    maps, qbias = _prep_inputs(**inputs)
    key = ("nc", qbias)
    if key not in _CACHE:
        _CACHE[key] = _build(qbias)
    _CACHE["nc"] = _CACHE[key]
    nc = _CACHE[key]
    res = run_bass_kernel_spmd(nc, maps, list(range(N_CORES)))
    out = np.empty((B, S, DIM), np.float32)
    for c in range(N_CORES):
        y = res.results[c]["y"]            # [DIM, 2*WC*QT]
        yv = y.reshape(DIM, QT, 2, WC)
        for b2 in range(2):
            for qt in range(QT):
                out[b2, 512 * qt + WC * c:512 * qt + WC * c + WC, :] = \
                    yv[:, qt, b2, :].T
    return out


# revision 58
# speedup vs baseline: 1.1290x; 1.1290x over previous
"""Trainium2 Bass kernel for pre-norm causal attention block.

Module: out = x + Wo(attn(LN(x))) with fused QKV, 16 heads, causal mask.
Shapes (hardcoded): x [2, 2048, 1024], wqkv [1024, 3072], wo [1024, 1024].

Sharding (8 cores, one program SPMD):
  core c = 4*b + s handles batch b, global heads [4s, 4s+4).  The attention
  context is exchanged with 4 small per-qt AllToAlls (64-token sub-chunks:
  core r owns tokens {512*qt + 64*r + i}), each overlapped with the next
  qt's attention compute; the receiver side does the output projection per
  chunk as it lands.

Per-core dataflow (feature-on-partitions, transposed):
  1. LN stats via ones-matmul on PE; fast-rsqrt NR on DVE.  LN mean
     correction is folded into the projections as an extra rank-1/2 matmul
     (lhsT = [-C; b2], rhs = [mu; 1]); LN scale r is applied as one
     tensor mult on Q, folded into the exp scale (r_k/8, per-partition AP)
     on K, and one tensor_scalar on V.  K/V biases are exact-folded
     (K bias cancels in softmax; V bias folded into bo on host).
  2. Scores per head-pair into one [128, 1024] PSUM tile, single exp per
     pair (split + masked on diagonal tiles), ctx accumulated per head in
     [65, 512] PSUM (row 64 = softmax denominator).
  3. Sender-side normalize: reciprocal_approx_fast on the 4 den rows,
     PE broadcast, one DVE mult -> normalized bf16 ctx^T; staged and
     shipped via the per-qt AllToAll (Shared output buffers).
  4. Receiver (interleaved per call): gather 8x[128,64] ctx blocks per
     batch in one DMA, output projection + residual + bias, store.
"""

import sys

for _p in ("/opt/trn_rl_repo",):
    if _p not in sys.path:
        sys.path.insert(0, _p)

import ml_dtypes
import numpy as np

import concourse.bass as bass
import concourse.mybir as mybir
import concourse.tile as tile
from concourse import bacc
from concourse.bass_utils import run_bass_kernel_spmd

F32 = mybir.dt.float32
F32R = mybir.dt.float32r
BF16 = mybir.dt.bfloat16
I32 = mybir.dt.int32
AF = mybir.ActivationFunctionType
ALU = mybir.AluOpType

N_CORES = 8
B, S, H, D = 2, 2048, 16, 64
DIM = H * D              # 1024
HL = 4                   # heads per core
DL = HL * D              # 256 local head features
WC = 64                  # per-call sub-chunk width (tokens)
EPS = 1e-6
KT = 128                 # k-tile (partition) width
NT = 512                 # matmul free-dim tile
FT = DIM // KT           # 8 feature tiles
ST = S // KT             # 16 seq tiles of 128
QT = S // NT             # 4 q-tiles of 512

_CACHE = {}


def _build(with_qbias):
    nc = bacc.Bacc("TRN2", target_bir_lowering=False, debug=False,
                   num_devices=N_CORES)

    # ---- I/O ----
    xbf_d = nc.dram_tensor("xbf", [DIM, S], BF16, kind="ExternalInput")
    xres_d = nc.dram_tensor("xres", [DIM, 2 * WC * QT], F32,
                            kind="ExternalInput")
    wqk_d = nc.dram_tensor("wqk", [DIM, 2 * DL], BF16, kind="ExternalInput")
    wv_d = nc.dram_tensor("wv", [DIM, DL], BF16, kind="ExternalInput")
    wo_d = nc.dram_tensor("wo", [DIM, DIM], BF16, kind="ExternalInput")
    augq_d = nc.dram_tensor("augq", [2, 2 * DL], BF16, kind="ExternalInput")
    ncv_d = nc.dram_tensor("ncv", [1, DL], BF16, kind="ExternalInput")
    bq_d = nc.dram_tensor("bq", [128, 2], F32, kind="ExternalInput")
    sel_d = nc.dram_tensor("sel", [2, 128], BF16, kind="ExternalInput")
    tri_d = nc.dram_tensor("tri", [128, 128], BF16, kind="ExternalInput")
    bo_d = nc.dram_tensor("bo_col", [128, FT], F32, kind="ExternalInput")
    y_d = nc.dram_tensor("y", [DIM, 2 * WC * QT], F32, kind="ExternalOutput")

    # ---- DRAM scratch ----
    stats_dram = nc.dram_tensor("stats_dram", [2, S], F32)
    a2a_in = [nc.dram_tensor(f"a2a_in{t}", [N_CORES, DL, WC], BF16)
              for t in range(QT)]
    a2a_out = [nc.dram_tensor(f"a2a_out{t}", [N_CORES, DL, WC], BF16)
               for t in range(QT)]

    with tile.TileContext(nc) as tc:
        import contextlib
        with contextlib.ExitStack() as ctx:
            _build_body(ctx, tc, nc, locals(), with_qbias)
    nc.compile()
    return nc


def _build_body(ctx, tc, nc, t, with_qbias):
    import math
    xbf_d, xres_d, wqk_d, wv_d, wo_d = (t["xbf_d"], t["xres_d"], t["wqk_d"],
                                        t["wv_d"], t["wo_d"])
    augq_d, ncv_d, bq_d, tri_d, bo_d, y_d = (
        t["augq_d"], t["ncv_d"], t["bq_d"], t["tri_d"], t["bo_d"], t["y_d"])
    sel_d = t["sel_d"]
    stats_dram, a2a_in, a2a_out = t["stats_dram"], t["a2a_in"], t["a2a_out"]

    P = 128
    sing = ctx.enter_context(tc.tile_pool(name="sing", bufs=1))
    # persistent SBUF tiles
    xbf = [sing.tile([P, S], BF16, tag=f"xbf{i}", name=f"xbf{i}")
           for i in range(FT)]
    xres = [sing.tile([P, 2 * WC * QT], F32, tag=f"xres{i}", name=f"xres{i}")
            for i in range(FT)]
    wqk = [sing.tile([P, 2 * DL], BF16, tag=f"wqk{i}", name=f"wqk{i}")
           for i in range(FT)]
    wv = [sing.tile([P, DL], BF16, tag=f"wv{i}", name=f"wv{i}")
          for i in range(FT)]
    wo = [sing.tile([P, DIM], BF16, tag=f"wo{i}", name=f"wo{i}")
          for i in range(FT)]
    qkT = [sing.tile([P, S], BF16, tag=f"qkT{i}", name=f"qkT{i}")
           for i in range(4)]
    vaug = [sing.tile([P, HL * (D + 1)], BF16, tag=f"vaug{i}",
                      name=f"vaug{i}") for i in range(ST)]
    rB = [sing.tile([P, NT], F32, tag=f"rB{i}", name=f"rB{i}")
          for i in range(QT)]
    augq = sing.tile([2, 2 * DL], BF16, tag="augq")
    ncv = sing.tile([1, DL], BF16, tag="ncv")
    bq_c = sing.tile([P, 2], F32, tag="bq")
    sel_a = sing.tile([1, P], BF16, tag="sel_a")
    sel_b = sing.tile([1, P], BF16, tag="sel_b")
    tri = sing.tile([P, P], BF16, tag="tri")
    bo_c = sing.tile([P, FT], F32, tag="bo")
    ones = sing.tile([P, 1], BF16, tag="ones")
    ones1 = sing.tile([1, P], BF16, tag="ones1")
    mu2 = sing.tile([2, S], BF16, tag="mu2")
    r_row = sing.tile([1, S], BF16, tag="r_row")
    sgP = sing.tile([P, ST], F32, tag="sgP")
    rcP = sing.tile([P, ST], F32, tag="rcP")
    idn = sing.tile([P, P], F32, tag="idn")

    # input DMAs -- xbf first (stats critical path), weights next, rest last
    for i in range(FT):
        nc.sync.dma_start(out=xbf[i], in_=xbf_d[i * P:(i + 1) * P, :])
    for i in range(FT):
        nc.sync.dma_start(out=wqk[i], in_=wqk_d[i * P:(i + 1) * P, :])
    for i in range(FT):
        nc.sync.dma_start(out=wv[i], in_=wv_d[i * P:(i + 1) * P, :])
    nc.sync.dma_start(out=augq, in_=augq_d[:])
    nc.sync.dma_start(out=ncv, in_=ncv_d[:])
    nc.sync.dma_start(out=bq_c, in_=bq_d[:])
    nc.sync.dma_start(out=sel_a, in_=sel_d[0:1, :])
    nc.sync.dma_start(out=sel_b, in_=sel_d[1:2, :])
    nc.sync.dma_start(out=tri, in_=tri_d[:])
    nc.sync.dma_start(out=bo_c, in_=bo_d[:])
    for i in range(FT):
        nc.sync.dma_start(out=wo[i], in_=wo_d[i * P:(i + 1) * P, :])
        nc.sync.dma_start(out=xres[i], in_=xres_d[i * P:(i + 1) * P, :])
    nc.vector.memset(ones, 1.0)
    nc.vector.memset(ones1, 1.0)
    nc.vector.memset(mu2, 1.0)       # row 0 overwritten by mu DMA below
    from concourse.masks import make_identity
    make_identity(nc, idn)

    # ---- 1. LN stats: column sums of x and x^2 via ones-matmul ----
    with tc.tile_pool(name="ps_st", bufs=4, space="PSUM") as ps_st, \
         tc.tile_pool(name="sqp", bufs=2) as sqp:
        stats_sa = sqp.tile([1, S], F32, tag="stats_sa", bufs=1)
        stats_sq = sqp.tile([1, S], F32, tag="stats_sq", bufs=1)
        sps = [ps_st.tile([1, NT], F32, tag="sum", name=f"sum{nt}")
               for nt in range(QT)]
        qps = [ps_st.tile([1, NT], F32, tag="sq", name=f"sqp{nt}")
               for nt in range(QT)]
        for k in range(FT):
            sq = sqp.tile([P, S], BF16, tag="sq", name="sq")
            nc.vector.tensor_mul(sq, xbf[k], xbf[k])
            for nt in range(QT):
                sl = slice(nt * NT, (nt + 1) * NT)
                nc.tensor.matmul(sps[nt], ones, xbf[k][:, sl],
                                 start=(k == 0), stop=(k == FT - 1))
                nc.tensor.matmul(qps[nt], ones, sq[:, sl],
                                 start=(k == 0), stop=(k == FT - 1))
        for nt in range(QT):
            sl = slice(nt * NT, (nt + 1) * NT)
            nc.vector.tensor_copy(stats_sa[:, sl], sps[nt])
            nc.vector.tensor_copy(stats_sq[:, sl], qps[nt])
    # QK main matmuls for mt=0 hoisted here: keeps PE busy during the
    # stats DRAM bounce + rsqrt chain (their aug/epilogue comes later).
    import contextlib as _ctl
    qk_ctx = _ctl.ExitStack()
    ps_qk = qk_ctx.enter_context(
        tc.tile_pool(name="ps_qk", bufs=5, space="PSUM"))
    tmp = qk_ctx.enter_context(tc.tile_pool(name="tmp", bufs=3))
    pre = []
    for nt in range(QT):
        ps = ps_qk.tile([P, NT], F32, tag="qk", name="qk")
        for k in range(FT):
            nc.tensor.matmul(ps, wqk[k][:, 0:P],
                             xbf[k][:, nt * NT:(nt + 1) * NT],
                             start=(k == 0), stop=False)
        pre.append(ps)
    nc.sync.dma_start(out=stats_dram[0:1], in_=stats_sa[:])
    nc.sync.dma_start(out=stats_dram[1:2], in_=stats_sq[:])
    # [16,128] reads, math at 16 partitions, then flatten (SBUF->SBUF DMA)
    sPT = sing.tile([16, P], F32, tag="sPT")
    qPT = sing.tile([16, P], F32, tag="qPT")
    nc.sync.dma_start(out=sPT, in_=stats_dram[0].rearrange("(j p) -> j p",
                                                           j=16))
    nc.sync.dma_start(out=qPT, in_=stats_dram[1].rearrange("(j p) -> j p",
                                                           j=16))
    muT = sing.tile([16, P], F32, tag="muT")
    nc.vector.tensor_scalar(muT, sPT, 1.0 / DIM, None, op0=ALU.mult)
    nc.vector.tensor_scalar(qPT, qPT, 1.0 / DIM, None, op0=ALU.mult)
    t0 = sing.tile([16, P], F32, tag="t0")
    nc.vector.tensor_mul(t0, muT, muT)
    nc.vector.tensor_sub(t0, qPT, t0)
    nc.vector.tensor_scalar(t0, t0, EPS, None, op0=ALU.add)
    # rsqrt via fast-inverse-square-root seed + 3 Newton steps
    rT = sing.tile([16, P], F32, tag="rT")
    t1s = sing.tile([16, P], F32, tag="t1s")
    nc.vector.tensor_scalar(rT[:].bitcast(I32), t0[:].bitcast(I32), 1, None,
                            op0=ALU.logical_shift_right)
    nc.vector.tensor_scalar(rT[:].bitcast(I32), rT[:].bitcast(I32), -1, None,
                            op0=ALU.bitwise_xor)
    nc.vector.tensor_scalar(rT[:].bitcast(I32), rT[:].bitcast(I32),
                            0x5F3759E0, None, op0=ALU.add)
    for _ in range(3):
        nc.vector.tensor_mul(t1s, rT, rT)
        nc.vector.tensor_mul(t1s, t1s, t0)
        nc.vector.tensor_scalar(t1s, t1s, -0.5, 1.5, op0=ALU.mult,
                                op1=ALU.add)
        nc.vector.tensor_mul(rT, rT, t1s)
    muTb = sing.tile([16, P], BF16, tag="muTb")
    nc.vector.tensor_copy(muTb, muT)
    nc.sync.dma_start(out=mu2[0:1, :], in_=muTb[:])
    rTb = sing.tile([16, P], BF16, tag="rTb")
    nc.vector.tensor_copy(rTb, rT)
    nc.sync.dma_start(out=r_row, in_=rTb[:])
    with tc.tile_pool(name="ps_bc", bufs=1, space="PSUM") as ps_bc:
        for nt in range(QT):
            sl = slice(nt * NT, (nt + 1) * NT)
            bp = ps_bc.tile([P, NT], F32, tag="bc", name="bc")
            nc.tensor.matmul(bp, ones1, r_row[:, sl], start=True, stop=True)
            nc.vector.tensor_copy(rB[nt], bp)
        # column layout via PE transpose: rcP (V epilogue), sgP (exp scale)
        tp = ps_bc.tile([P, 16], F32, tag="tp", name="tp")
        nc.tensor.transpose(tp, rT[:], idn[0:16, 0:16])
        nc.vector.tensor_copy(rcP, tp)
        nc.vector.tensor_scalar(sgP, rcP, 1.0 / math.sqrt(D), None,
                                op0=ALU.mult)

    # ---- 2. QK projection ----
    if True:
        for mt in range(4):          # qkT M-tiles (Q01 Q23 K01 K23)
            for nt in range(QT):
                sl = slice(nt * NT, (nt + 1) * NT)
                if mt == 0:
                    ps = pre[nt]
                else:
                    ps = ps_qk.tile([P, NT], F32, tag="qk", name="qk")
                    for k in range(FT):
                        nc.tensor.matmul(
                            ps, wqk[k][:, mt * P:(mt + 1) * P],
                            xbf[k][:, sl], start=(k == 0), stop=False)
                nc.tensor.matmul(ps, augq[:, mt * P:(mt + 1) * P],
                                 mu2[:, sl], start=False, stop=True)
                if mt < 2:
                    # Q: apply LN scale r (per-token broadcast)
                    if with_qbias:
                        t1 = tmp.tile([P, NT], F32, tag="t1")
                        nc.vector.tensor_mul(t1, ps, rB[nt])
                        nc.vector.tensor_scalar(
                            qkT[mt][:, sl], t1, bq_c[:, mt:mt + 1], None,
                            op0=ALU.add)
                    else:
                        nc.vector.tensor_mul(qkT[mt][:, sl], ps, rB[nt])
                else:
                    # K: r folded into exp scale; plain copy to bf16
                    nc.scalar.copy(qkT[mt][:, sl], ps)

    qk_ctx.close()

    # ---- 4. attention + per-qt A2A + interleaved receiver ----
    # (V projection woven in per-qt: tile st is produced just before the
    #  first q-tile that needs it, borrowing the scores PSUM ring.)
    with tc.tile_pool(name="ps_sc", bufs=2, space="PSUM") as ps_sc, \
         tc.tile_pool(name="ps_cx", bufs=1, space="PSUM") as ps_cx, \
         tc.tile_pool(name="esp", bufs=4) as esp, \
         tc.tile_pool(name="ctxp", bufs=4) as ctxp, \
         tc.tile_pool(name="denp", bufs=2) as denp, \
         tc.tile_pool(name="cap", bufs=4) as cap, \
         tc.tile_pool(name="yp", bufs=4) as yp:

        def receiver(call):
            # gather ctx blocks: one DMA per batch
            ca = []
            for b2 in range(2):
                cat = cap.tile([P, FT, WC], BF16, tag="ca", name="ca")
                nc.sync.dma_start(
                    out=cat[:],
                    in_=a2a_out[call][4 * b2:4 * b2 + 4].rearrange(
                        "g (f p) q -> p (g f) q", f=2))
                ca.append(cat)
            for mt in range(FT):
                pof = ps_sc.tile([P, 2 * NT], F32, tag="sc", name="po")
                for k in range(FT):
                    for b2 in range(2):
                        nc.tensor.matmul(
                            pof[:, b2 * NT:b2 * NT + WC],
                            wo[k][:, mt * P:(mt + 1) * P],
                            ca[b2][:, k, :],
                            start=(k == 0), stop=(k == FT - 1))
                ysb = yp.tile([P, 2 * WC], F32, tag="ysb", name="ysb")
                yout = yp.tile([P, 2 * WC], F32, tag="yout", name="yout")
                csl = slice(call * 2 * WC, (call + 1) * 2 * WC)
                for b2 in range(2):
                    nc.vector.tensor_add(
                        ysb[:, b2 * WC:(b2 + 1) * WC],
                        pof[:, b2 * NT:b2 * NT + WC],
                        xres[mt][:, call * 2 * WC + b2 * WC:
                                 call * 2 * WC + (b2 + 1) * WC])
                nc.scalar.activation(yout, ysb, AF.Identity,
                                     bias=bo_c[:, mt:mt + 1])
                nc.sync.dma_start(out=y_d[mt * P:(mt + 1) * P, csl],
                                  in_=yout)

        def emit_v(st):
            psf = ps_sc.tile([P, 2 * NT], F32, tag="sc", name="v")
            ps = psf[:, 0:DL]
            for k in range(FT):
                nc.tensor.matmul(
                    ps, xbf[k][:, st * P:(st + 1) * P], wv[k],
                    start=(k == 0), stop=False)
            nc.tensor.matmul(ps, mu2[0:1, st * P:(st + 1) * P], ncv,
                             start=False, stop=True)
            nc.vector.tensor_scalar(
                vaug[st][:].rearrange("p (h e) -> p h e", h=HL)[:, :, 0:D],
                ps.rearrange("p (h d) -> p h d", h=HL),
                rcP[:, st:st + 1], None, op0=ALU.mult)
            nc.vector.memset(
                vaug[st][:].rearrange("p (h e) -> p h e", h=HL)[:, :,
                                                                D:D + 1],
                1.0)

        for qt in range(QT):
            for st in range(4 * qt, 4 * qt + 4):
                emit_v(st)
            q0 = qt * NT
            cxs = [ps_cx.tile([D + 1, NT], F32, tag=f"cx{hl}",
                              name=f"cx{hl}") for hl in range(HL)]
            def emit_ctx(es_pair, kt):
                for pr in range(2):
                    for u in range(2):
                        hl = 2 * pr + u
                        nc.tensor.matmul(
                            cxs[hl],
                            vaug[kt][:, hl * (D + 1):(hl + 1) * (D + 1)],
                            es_pair[pr][:, u * NT:(u + 1) * NT],
                            start=(kt == 0), stop=(kt == 4 * qt + 3))

            pend = None            # (es_pair, kt) deferred by one k-tile
            for kt in range(4 * qt + 4):
                k0 = kt * KT
                dlt = k0 - q0          # >0 only on diagonal k-tiles
                cur = []
                for pr in range(2):    # head pairs (2pr, 2pr+1)
                    sc = ps_sc.tile([P, 2 * NT], F32, tag="sc", name="sc")
                    es = esp.tile([P, 2 * NT], BF16, tag="es", name="es")
                    for u in range(2):
                        hp = slice(D * u, D * u + D)
                        off = u * NT
                        if dlt > 0:
                            nc.vector.memset(es[:, off:off + dlt], 0.0)
                            nc.tensor.matmul(
                                sc[:, off + dlt:off + NT],
                                qkT[2 + pr][hp, k0:k0 + KT],
                                qkT[pr][hp, q0 + dlt:q0 + NT],
                                start=True, stop=True)
                        else:
                            nc.tensor.matmul(
                                sc[:, off:off + NT],
                                qkT[2 + pr][hp, k0:k0 + KT],
                                qkT[pr][hp, q0:q0 + NT],
                                start=True, stop=True)
                    if dlt > 0:
                        for u in range(2):
                            off = u * NT
                            nc.scalar.activation(
                                es[:, off + dlt:off + NT],
                                sc[:, off + dlt:off + NT], AF.Exp,
                                scale=sgP[:, kt:kt + 1])
                    else:
                        nc.scalar.activation(es, sc, AF.Exp,
                                             scale=sgP[:, kt:kt + 1])
                    if dlt >= 0 and kt >= 4 * qt:   # diagonal triangle
                        for u in range(2):
                            off = u * NT
                            nc.vector.tensor_mul(
                                es[:, off + dlt:off + dlt + KT],
                                es[:, off + dlt:off + dlt + KT], tri)
                    cur.append(es)
                if pend is not None:
                    emit_ctx(*pend)
                pend = (cur, kt)
            emit_ctx(*pend)
            # qt tail: sender-side normalize + stage + collective
            dens = denp.tile([1, HL * NT], BF16, tag="dens", name="dens")
            for hl in range(HL):
                nc.scalar.copy(dens[:, hl * NT:(hl + 1) * NT],
                               cxs[hl][D:D + 1, :])
            for pr in range(2):
                rbt = ps_sc.tile([P, 2 * NT], F32, tag="sc", name="rb")
                for u in range(2):
                    hl = 2 * pr + u
                    nc.tensor.matmul(
                        rbt[:, 0:NT],
                        sel_a if u == 0 else sel_b,
                        dens[:, hl * NT:(hl + 1) * NT],
                        start=(u == 0), stop=(u == 1))
                dsb = denp.tile([P, NT], F32, tag="dsb", name="dsb")
                nc.vector.tensor_copy(dsb, rbt[:, 0:NT])
                rcb = denp.tile([P, NT], F32, tag="rcb", name="rcb")
                nc.vector.reciprocal_approx_fast(out=rcb[:], in_=dsb[:])
                for u in range(2):
                    hl = 2 * pr + u
                    ct = ctxp.tile([D, NT], BF16, tag="ct", name="ct")
                    nc.vector.tensor_mul(ct, cxs[hl][0:D, :],
                                         rcb[u * D:(u + 1) * D, :])
                    nc.sync.dma_start(
                            out=a2a_in[qt][:, hl * D:(hl + 1) * D,
                                           :].rearrange("d p q -> p d q"),
                            in_=ct[:].rearrange("p (d q) -> p d q",
                                                d=N_CORES))
            nc.gpsimd.collective_compute(
                "AllToAll", ALU.bypass,
                replica_groups=[list(range(N_CORES))],
                ins=[a2a_in[qt][:].opt()], outs=[a2a_out[qt][:].opt()],
                unique_tensors="Yes")
            if qt >= 1:
                receiver(qt - 1)
        receiver(QT - 1)


def _prep_inputs(x, ln_g, ln_b, wqkv, bqkv, wo, bo):
    """Host-side sharding / folding. Returns per-core input dicts."""
    f32 = np.float32
    bf16 = ml_dtypes.bfloat16
    x = np.asarray(x, f32)
    wg = (np.asarray(wqkv, f32) * np.asarray(ln_g, f32)[:, None])
    tri = (np.arange(128)[None, :] >= np.arange(128)[:, None]).astype(bf16)
    wo_f = np.asarray(wo, f32)
    wo_bf = wo_f.astype(bf16)
    lnb = np.asarray(ln_b, f32)
    bq = np.asarray(bqkv, f32)
    bo_f = np.asarray(bo, f32)

    xT = [np.ascontiguousarray(x[b].T) for b in range(B)]
    xbf = [t.astype(bf16) for t in xT]

    # V bias folded through Wo: full ctx bias vector (all head groups)
    b2v_full = np.zeros(DIM, f32)
    for s in range(4):
        vs = slice(2 * DIM + DL * s, 2 * DIM + DL * s + DL)
        wv_f = wg[:, vs]
        b2v_full[DL * s:DL * s + DL] = bq[vs] + wv_f.T @ lnb
    bo2 = bo_f + b2v_full @ wo_f
    sel2 = np.zeros((2, 128), np.float32)
    sel2[0, 0:64] = 1.0
    sel2[1, 64:128] = 1.0
    sel2 = sel2.astype(bf16)

    maps = []
    qbias = False
    for c in range(N_CORES):
        b, s = divmod(c, 4)
        qs = slice(DL * s, DL * s + DL)
        ks = slice(DIM + DL * s, DIM + DL * s + DL)
        vs = slice(2 * DIM + DL * s, 2 * DIM + DL * s + DL)
        wqk_l = np.concatenate([wg[:, qs], wg[:, ks]], axis=1).astype(bf16)
        wv_l = wg[:, vs].astype(bf16)
        wqk_f = wqk_l.astype(f32)
        wv_f = wv_l.astype(f32)
        cqk = wqk_f.sum(0)                       # [512]
        b2q = bq[qs] + wqk_f[:, 0:DL].T @ lnb    # Q bias (post-scale ref!)
        cv = wv_f.sum(0)                         # [256]
        if np.abs(b2q).max() > 0:
            qbias = True
        augq = np.stack([-cqk, np.zeros(2 * DL, f32)]).astype(bf16)
        # tokens for core c: 512*qt + 64*c + i, cols ordered [qt][b2][64]
        toks = (512 * np.arange(QT)[:, None] + WC * c
                + np.arange(WC)[None, :]).reshape(-1)
        xres_c = np.stack([xT[b2][:, toks] for b2 in range(2)], axis=1)
        xres_c = xres_c.reshape(DIM, 2, QT, WC).transpose(0, 2, 1, 3)
        xres_c = np.ascontiguousarray(xres_c.reshape(DIM, 2 * WC * QT))
        maps.append({
            "xbf": xbf[b],
            "xres": xres_c,
            "wqk": wqk_l,
            "wv": wv_l,
            "wo": wo_bf,
            "augq": augq,
            "ncv": np.ascontiguousarray(-cv[None, :]).astype(bf16),
            "bq": np.ascontiguousarray(
                b2q.reshape(2, 128).T.astype(f32)),
            "tri": tri,
            "bo_col": np.ascontiguousarray(bo2.reshape(FT, 128).T),
            "sel": sel2,
        })
    maps_qbias = qbias
    return maps, maps_qbias


def kernel(**inputs):
    maps, qbias = _prep_inputs(**inputs)
    key = ("nc", qbias)
    if key not in _CACHE:
        _CACHE[key] = _build(qbias)
    _CACHE["nc"] = _CACHE[key]
    nc = _CACHE[key]
    res = run_bass_kernel_spmd(nc, maps, list(range(N_CORES)))
    out = np.empty((B, S, DIM), np.float32)
    for c in range(N_CORES):
        y = res.results[c]["y"]            # [DIM, 2*WC*QT]
        yv = y.reshape(DIM, QT, 2, WC)
        for b2 in range(2):
            for qt in range(QT):
                out[b2, 512 * qt + WC * c:512 * qt + WC * c + WC, :] = \
                    yv[:, qt, b2, :].T
    return out


# revision 59
# speedup vs baseline: 1.1911x; 1.0550x over previous
"""Trainium2 Bass kernel for pre-norm causal attention block.

Module: out = x + Wo(attn(LN(x))) with fused QKV, 16 heads, causal mask.
Shapes (hardcoded): x [2, 2048, 1024], wqkv [1024, 3072], wo [1024, 1024].

Sharding (8 cores, one program SPMD):
  core c = 4*b + s handles batch b, global heads [4s, 4s+4).  The attention
  context is exchanged with 4 small per-qt AllToAlls (64-token sub-chunks:
  core r owns tokens {512*qt + 64*r + i}), each overlapped with the next
  qt's attention compute; the receiver side does the output projection per
  chunk as it lands.

Per-core dataflow (feature-on-partitions, transposed):
  1. LN stats via ones-matmul on PE; fast-rsqrt NR on DVE.  LN mean
     correction is folded into the projections as an extra rank-1/2 matmul
     (lhsT = [-C; b2], rhs = [mu; 1]); LN scale r is applied as one
     tensor mult on Q, folded into the exp scale (r_k/8, per-partition AP)
     on K, and one tensor_scalar on V.  K/V biases are exact-folded
     (K bias cancels in softmax; V bias folded into bo on host).
  2. Scores per head-pair into one [128, 1024] PSUM tile, single exp per
     pair (split + masked on diagonal tiles), ctx accumulated per head in
     [65, 512] PSUM (row 64 = softmax denominator).
  3. Sender-side normalize: reciprocal_approx_fast on the 4 den rows,
     PE broadcast, one DVE mult -> normalized bf16 ctx^T; staged and
     shipped via the per-qt AllToAll (Shared output buffers).
  4. Receiver (interleaved per call): gather 8x[128,64] ctx blocks per
     batch in one DMA, output projection + residual + bias, store.
"""

import sys

for _p in ("/opt/trn_rl_repo",):
    if _p not in sys.path:
        sys.path.insert(0, _p)

import ml_dtypes
import numpy as np

import concourse.bass as bass
import concourse.mybir as mybir
import concourse.tile as tile
from concourse import bacc
from concourse.bass_utils import run_bass_kernel_spmd

F32 = mybir.dt.float32
F32R = mybir.dt.float32r
BF16 = mybir.dt.bfloat16
I32 = mybir.dt.int32
AF = mybir.ActivationFunctionType
ALU = mybir.AluOpType

N_CORES = 8
B, S, H, D = 2, 2048, 16, 64
DIM = H * D              # 1024
HL = 4                   # heads per core
DL = HL * D              # 256 local head features
WC = 64                  # per-call sub-chunk width (tokens)
EPS = 1e-6
KT = 128                 # k-tile (partition) width
NT = 512                 # matmul free-dim tile
FT = DIM // KT           # 8 feature tiles
ST = S // KT             # 16 seq tiles of 128
QT = S // NT             # 4 q-tiles of 512

_CACHE = {}


def _build(with_qbias):
    nc = bacc.Bacc("TRN2", target_bir_lowering=False, debug=False,
                   num_devices=N_CORES)

    # ---- I/O ----
    xbf_d = nc.dram_tensor("xbf", [DIM, S], BF16, kind="ExternalInput")
    xres_d = nc.dram_tensor("xres", [DIM, 2 * WC * QT], F32,
                            kind="ExternalInput")
    wqk_d = nc.dram_tensor("wqk", [DIM, 2 * DL], BF16, kind="ExternalInput")
    wv_d = nc.dram_tensor("wv", [DIM, DL], BF16, kind="ExternalInput")
    wo_d = nc.dram_tensor("wo", [DIM, DIM], BF16, kind="ExternalInput")
    augq_d = nc.dram_tensor("augq", [2, 2 * DL], BF16, kind="ExternalInput")
    ncv_d = nc.dram_tensor("ncv", [1, DL], BF16, kind="ExternalInput")
    bq_d = nc.dram_tensor("bq", [128, 2], F32, kind="ExternalInput")
    sel_d = nc.dram_tensor("sel", [2, 128], BF16, kind="ExternalInput")
    tri_d = nc.dram_tensor("tri", [128, 128], BF16, kind="ExternalInput")
    bo_d = nc.dram_tensor("bo_col", [128, FT], F32, kind="ExternalInput")
    y_d = nc.dram_tensor("y", [DIM, 2 * WC * QT], F32, kind="ExternalOutput")

    # ---- DRAM scratch ----
    stats_dram = nc.dram_tensor("stats_dram", [2, S], F32)
    a2a_in = [nc.dram_tensor(f"a2a_in{t}", [N_CORES, DL, WC], BF16)
              for t in range(QT)]
    a2a_out = [nc.dram_tensor(f"a2a_out{t}", [N_CORES, DL, WC], BF16)
               for t in range(QT)]

    with tile.TileContext(nc) as tc:
        import contextlib
        with contextlib.ExitStack() as ctx:
            _build_body(ctx, tc, nc, locals(), with_qbias)
    nc.compile()
    return nc


def _build_body(ctx, tc, nc, t, with_qbias):
    import math
    xbf_d, xres_d, wqk_d, wv_d, wo_d = (t["xbf_d"], t["xres_d"], t["wqk_d"],
                                        t["wv_d"], t["wo_d"])
    augq_d, ncv_d, bq_d, tri_d, bo_d, y_d = (
        t["augq_d"], t["ncv_d"], t["bq_d"], t["tri_d"], t["bo_d"], t["y_d"])
    sel_d = t["sel_d"]
    stats_dram, a2a_in, a2a_out = t["stats_dram"], t["a2a_in"], t["a2a_out"]

    P = 128
    sing = ctx.enter_context(tc.tile_pool(name="sing", bufs=1))
    # persistent SBUF tiles
    xbf = [sing.tile([P, S], BF16, tag=f"xbf{i}", name=f"xbf{i}")
           for i in range(FT)]
    xres = [sing.tile([P, 2 * WC * QT], F32, tag=f"xres{i}", name=f"xres{i}")
            for i in range(FT)]
    wqk = [sing.tile([P, 2 * DL], BF16, tag=f"wqk{i}", name=f"wqk{i}")
           for i in range(FT)]
    wv = [sing.tile([P, DL], BF16, tag=f"wv{i}", name=f"wv{i}")
          for i in range(FT)]
    wo = [sing.tile([P, DIM], BF16, tag=f"wo{i}", name=f"wo{i}")
          for i in range(FT)]
    qkT = [sing.tile([P, S], BF16, tag=f"qkT{i}", name=f"qkT{i}")
           for i in range(4)]
    vaug = [sing.tile([P, HL * (D + 1)], BF16, tag=f"vaug{i}",
                      name=f"vaug{i}") for i in range(ST)]
    rB = [sing.tile([P, NT], F32, tag=f"rB{i}", name=f"rB{i}")
          for i in range(QT)]
    augq = sing.tile([2, 2 * DL], BF16, tag="augq")
    ncv = sing.tile([1, DL], BF16, tag="ncv")
    bq_c = sing.tile([P, 2], F32, tag="bq")
    sel_a = sing.tile([1, P], BF16, tag="sel_a")
    sel_b = sing.tile([1, P], BF16, tag="sel_b")
    tri = sing.tile([P, P], BF16, tag="tri")
    bo_c = sing.tile([P, FT], F32, tag="bo")
    ones = sing.tile([P, 1], BF16, tag="ones")
    ones1 = sing.tile([1, P], BF16, tag="ones1")
    mu2 = sing.tile([2, S], BF16, tag="mu2")
    r_row = sing.tile([1, S], BF16, tag="r_row")
    sgP = sing.tile([P, ST], F32, tag="sgP")
    rcP = sing.tile([P, ST], F32, tag="rcP")
    idn = sing.tile([P, P], F32, tag="idn")

    # input DMAs -- xbf first (stats critical path), weights next, rest last
    for i in range(FT):
        nc.sync.dma_start(out=xbf[i], in_=xbf_d[i * P:(i + 1) * P, :])
    for i in range(FT):
        nc.sync.dma_start(out=wqk[i], in_=wqk_d[i * P:(i + 1) * P, :])
    for i in range(FT):
        nc.sync.dma_start(out=wv[i], in_=wv_d[i * P:(i + 1) * P, :])
    nc.sync.dma_start(out=augq, in_=augq_d[:])
    nc.sync.dma_start(out=ncv, in_=ncv_d[:])
    nc.sync.dma_start(out=bq_c, in_=bq_d[:])
    nc.sync.dma_start(out=sel_a, in_=sel_d[0:1, :])
    nc.sync.dma_start(out=sel_b, in_=sel_d[1:2, :])
    nc.sync.dma_start(out=tri, in_=tri_d[:])
    nc.sync.dma_start(out=bo_c, in_=bo_d[:])
    for i in range(FT):
        nc.sync.dma_start(out=wo[i], in_=wo_d[i * P:(i + 1) * P, :])
        nc.sync.dma_start(out=xres[i], in_=xres_d[i * P:(i + 1) * P, :])
    nc.vector.memset(ones, 1.0)
    nc.vector.memset(ones1, 1.0)
    nc.vector.memset(mu2, 1.0)       # row 0 overwritten by mu DMA below
    from concourse.masks import make_identity
    make_identity(nc, idn)

    # ---- 1. LN stats: column sums of x and x^2 via ones-matmul ----
    with tc.tile_pool(name="ps_st", bufs=4, space="PSUM") as ps_st, \
         tc.tile_pool(name="sqp", bufs=2) as sqp:
        stats_sa = sqp.tile([1, S], F32, tag="stats_sa", bufs=1)
        stats_sq = sqp.tile([1, S], F32, tag="stats_sq", bufs=1)
        sps = [ps_st.tile([1, NT], F32, tag="sum", name=f"sum{nt}")
               for nt in range(QT)]
        qps = [ps_st.tile([1, NT], F32, tag="sq", name=f"sqp{nt}")
               for nt in range(QT)]
        for k in range(FT):
            sq = sqp.tile([P, S], BF16, tag="sq", name="sq")
            nc.vector.tensor_mul(sq, xbf[k], xbf[k])
            for nt in range(QT):
                sl = slice(nt * NT, (nt + 1) * NT)
                nc.tensor.matmul(sps[nt], ones, xbf[k][:, sl],
                                 start=(k == 0), stop=(k == FT - 1))
                nc.tensor.matmul(qps[nt], ones, sq[:, sl],
                                 start=(k == 0), stop=(k == FT - 1))
        for nt in range(QT):
            sl = slice(nt * NT, (nt + 1) * NT)
            nc.vector.tensor_copy(stats_sa[:, sl], sps[nt])
            nc.vector.tensor_copy(stats_sq[:, sl], qps[nt])
    # QK main matmuls for mt=0 hoisted here: keeps PE busy during the
    # stats DRAM bounce + rsqrt chain (their aug/epilogue comes later).
    import contextlib as _ctl
    qk_ctx = _ctl.ExitStack()
    ps_qk = qk_ctx.enter_context(
        tc.tile_pool(name="ps_qk", bufs=5, space="PSUM"))
    tmp = qk_ctx.enter_context(tc.tile_pool(name="tmp", bufs=3))
    pre = []
    for nt in range(QT):
        ps = ps_qk.tile([P, NT], F32, tag="qk", name="qk")
        for k in range(FT):
            nc.tensor.matmul(ps, wqk[k][:, 0:P],
                             xbf[k][:, nt * NT:(nt + 1) * NT],
                             start=(k == 0), stop=False)
        pre.append(ps)
    nc.sync.dma_start(out=stats_dram[0:1], in_=stats_sa[:])
    nc.sync.dma_start(out=stats_dram[1:2], in_=stats_sq[:])
    # [16,128] reads, math at 16 partitions, then flatten (SBUF->SBUF DMA)
    sPT = sing.tile([16, P], F32, tag="sPT")
    qPT = sing.tile([16, P], F32, tag="qPT")
    nc.sync.dma_start(out=sPT, in_=stats_dram[0].rearrange("(j p) -> j p",
                                                           j=16))
    nc.sync.dma_start(out=qPT, in_=stats_dram[1].rearrange("(j p) -> j p",
                                                           j=16))
    muT = sing.tile([16, P], F32, tag="muT")
    nc.vector.tensor_scalar(muT, sPT, 1.0 / DIM, None, op0=ALU.mult)
    nc.vector.tensor_scalar(qPT, qPT, 1.0 / DIM, None, op0=ALU.mult)
    t0 = sing.tile([16, P], F32, tag="t0")
    nc.vector.tensor_mul(t0, muT, muT)
    nc.vector.tensor_sub(t0, qPT, t0)
    nc.vector.tensor_scalar(t0, t0, EPS, None, op0=ALU.add)
    # rsqrt via fast-inverse-square-root seed + 3 Newton steps
    rT = sing.tile([16, P], F32, tag="rT")
    t1s = sing.tile([16, P], F32, tag="t1s")
    nc.vector.tensor_scalar(rT[:].bitcast(I32), t0[:].bitcast(I32), 1, None,
                            op0=ALU.logical_shift_right)
    nc.vector.tensor_scalar(rT[:].bitcast(I32), rT[:].bitcast(I32), -1, None,
                            op0=ALU.bitwise_xor)
    nc.vector.tensor_scalar(rT[:].bitcast(I32), rT[:].bitcast(I32),
                            0x5F3759E0, None, op0=ALU.add)
    for _ in range(3):
        nc.vector.tensor_mul(t1s, rT, rT)
        nc.vector.tensor_mul(t1s, t1s, t0)
        nc.vector.tensor_scalar(t1s, t1s, -0.5, 1.5, op0=ALU.mult,
                                op1=ALU.add)
        nc.vector.tensor_mul(rT, rT, t1s)
    muTb = sing.tile([16, P], BF16, tag="muTb")
    nc.vector.tensor_copy(muTb, muT)
    nc.sync.dma_start(out=mu2[0:1, :], in_=muTb[:])
    rTb = sing.tile([16, P], BF16, tag="rTb")
    nc.vector.tensor_copy(rTb, rT)
    nc.sync.dma_start(out=r_row, in_=rTb[:])
    with tc.tile_pool(name="ps_bc", bufs=1, space="PSUM") as ps_bc:
        for nt in range(QT):
            sl = slice(nt * NT, (nt + 1) * NT)
            bp = ps_bc.tile([P, NT], F32, tag="bc", name="bc")
            nc.tensor.matmul(bp, ones1, r_row[:, sl], start=True, stop=True)
            nc.vector.tensor_copy(rB[nt], bp)
        # column layout via PE transpose: rcP (V epilogue), sgP (exp scale)
        tp = ps_bc.tile([P, 16], F32, tag="tp", name="tp")
        nc.tensor.transpose(tp, rT[:], idn[0:16, 0:16])
        nc.vector.tensor_copy(rcP, tp)
        nc.vector.tensor_scalar(sgP, rcP, 1.0 / math.sqrt(D), None,
                                op0=ALU.mult)

    # ---- 2. QK projection ----
    if True:
        for mt in range(4):          # qkT M-tiles (Q01 Q23 K01 K23)
            for nt in range(QT):
                sl = slice(nt * NT, (nt + 1) * NT)
                if mt == 0:
                    ps = pre[nt]
                else:
                    ps = ps_qk.tile([P, NT], F32, tag="qk", name="qk")
                    for k in range(FT):
                        nc.tensor.matmul(
                            ps, wqk[k][:, mt * P:(mt + 1) * P],
                            xbf[k][:, sl], start=(k == 0), stop=False)
                nc.tensor.matmul(ps, augq[:, mt * P:(mt + 1) * P],
                                 mu2[:, sl], start=False, stop=True)
                if mt < 2:
                    # Q: apply LN scale r (per-token broadcast)
                    if with_qbias:
                        t1 = tmp.tile([P, NT], F32, tag="t1")
                        nc.vector.tensor_mul(t1, ps, rB[nt])
                        nc.vector.tensor_scalar(
                            qkT[mt][:, sl], t1, bq_c[:, mt:mt + 1], None,
                            op0=ALU.add)
                    else:
                        nc.vector.tensor_mul(qkT[mt][:, sl], ps, rB[nt])
                else:
                    # K: r folded into exp scale; plain copy to bf16
                    nc.scalar.copy(qkT[mt][:, sl], ps)

    qk_ctx.close()

    # ---- 4. attention + per-qt A2A + interleaved receiver ----
    # (V projection woven in per-qt: tile st is produced just before the
    #  first q-tile that needs it, borrowing the scores PSUM ring.)
    with tc.tile_pool(name="ps_sc", bufs=2, space="PSUM") as ps_sc, \
         tc.tile_pool(name="ps_cx", bufs=1, space="PSUM") as ps_cx, \
         tc.tile_pool(name="esp", bufs=6) as esp, \
         tc.tile_pool(name="ctxp", bufs=4) as ctxp, \
         tc.tile_pool(name="denp", bufs=2) as denp, \
         tc.tile_pool(name="cap", bufs=4) as cap, \
         tc.tile_pool(name="yp", bufs=4) as yp:

        def receiver(call):
            # gather ctx blocks: one DMA per batch
            ca = []
            for b2 in range(2):
                cat = cap.tile([P, FT, WC], BF16, tag="ca", name="ca")
                nc.sync.dma_start(
                    out=cat[:],
                    in_=a2a_out[call][4 * b2:4 * b2 + 4].rearrange(
                        "g (f p) q -> p (g f) q", f=2))
                ca.append(cat)
            for mt in range(FT):
                pof = ps_sc.tile([P, 2 * NT], F32, tag="sc", name="po")
                for k in range(FT):
                    for b2 in range(2):
                        nc.tensor.matmul(
                            pof[:, b2 * NT:b2 * NT + WC],
                            wo[k][:, mt * P:(mt + 1) * P],
                            ca[b2][:, k, :],
                            start=(k == 0), stop=(k == FT - 1))
                ysb = yp.tile([P, 2 * WC], F32, tag="ysb", name="ysb")
                yout = yp.tile([P, 2 * WC], F32, tag="yout", name="yout")
                csl = slice(call * 2 * WC, (call + 1) * 2 * WC)
                for b2 in range(2):
                    nc.vector.tensor_add(
                        ysb[:, b2 * WC:(b2 + 1) * WC],
                        pof[:, b2 * NT:b2 * NT + WC],
                        xres[mt][:, call * 2 * WC + b2 * WC:
                                 call * 2 * WC + (b2 + 1) * WC])
                nc.scalar.activation(yout, ysb, AF.Identity,
                                     bias=bo_c[:, mt:mt + 1])
                nc.sync.dma_start(out=y_d[mt * P:(mt + 1) * P, csl],
                                  in_=yout)

        def emit_v(st):
            psf = ps_sc.tile([P, 2 * NT], F32, tag="sc", name="v")
            ps = psf[:, 0:DL]
            for k in range(FT):
                nc.tensor.matmul(
                    ps, xbf[k][:, st * P:(st + 1) * P], wv[k],
                    start=(k == 0), stop=False)
            nc.tensor.matmul(ps, mu2[0:1, st * P:(st + 1) * P], ncv,
                             start=False, stop=True)
            nc.vector.tensor_scalar(
                vaug[st][:].rearrange("p (h e) -> p h e", h=HL)[:, :, 0:D],
                ps.rearrange("p (h d) -> p h d", h=HL),
                rcP[:, st:st + 1], None, op0=ALU.mult)
            nc.vector.memset(
                vaug[st][:].rearrange("p (h e) -> p h e", h=HL)[:, :,
                                                                D:D + 1],
                1.0)

        for qt in range(QT):
            for st in range(4 * qt, 4 * qt + 4):
                emit_v(st)
            q0 = qt * NT
            cxs = [ps_cx.tile([D + 1, NT], F32, tag=f"cx{hl}",
                              name=f"cx{hl}") for hl in range(HL)]
            def emit_ctx(es_pair, kt):
                for pr in range(2):
                    for u in range(2):
                        hl = 2 * pr + u
                        nc.tensor.matmul(
                            cxs[hl],
                            vaug[kt][:, hl * (D + 1):(hl + 1) * (D + 1)],
                            es_pair[pr][:, u * NT:(u + 1) * NT],
                            start=(kt == 0), stop=(kt == 4 * qt + 3))

            pends = []             # (es_pair, kt) deferred 2 k-tiles
            for kt in range(4 * qt + 4):
                k0 = kt * KT
                dlt = k0 - q0          # >0 only on diagonal k-tiles
                cur = []
                for pr in range(2):    # head pairs (2pr, 2pr+1)
                    sc = ps_sc.tile([P, 2 * NT], F32, tag="sc", name="sc")
                    es = esp.tile([P, 2 * NT], BF16, tag="es", name="es")
                    for u in range(2):
                        hp = slice(D * u, D * u + D)
                        off = u * NT
                        if dlt > 0:
                            nc.vector.memset(es[:, off:off + dlt], 0.0)
                            nc.tensor.matmul(
                                sc[:, off + dlt:off + NT],
                                qkT[2 + pr][hp, k0:k0 + KT],
                                qkT[pr][hp, q0 + dlt:q0 + NT],
                                start=True, stop=True)
                        else:
                            nc.tensor.matmul(
                                sc[:, off:off + NT],
                                qkT[2 + pr][hp, k0:k0 + KT],
                                qkT[pr][hp, q0:q0 + NT],
                                start=True, stop=True)
                    if dlt > 0:
                        for u in range(2):
                            off = u * NT
                            nc.scalar.activation(
                                es[:, off + dlt:off + NT],
                                sc[:, off + dlt:off + NT], AF.Exp,
                                scale=sgP[:, kt:kt + 1])
                    else:
                        nc.scalar.activation(es, sc, AF.Exp,
                                             scale=sgP[:, kt:kt + 1])
                    if dlt >= 0 and kt >= 4 * qt:   # diagonal triangle
                        for u in range(2):
                            off = u * NT
                            nc.vector.tensor_mul(
                                es[:, off + dlt:off + dlt + KT],
                                es[:, off + dlt:off + dlt + KT], tri)
                    cur.append(es)
                pends.append((cur, kt))
                if len(pends) > 2:
                    emit_ctx(*pends.pop(0))
            for pc in pends:
                emit_ctx(*pc)
            # qt tail: sender-side normalize + stage + collective
            dens = denp.tile([1, HL * NT], BF16, tag="dens", name="dens")
            for hl in range(HL):
                nc.scalar.copy(dens[:, hl * NT:(hl + 1) * NT],
                               cxs[hl][D:D + 1, :])
            for pr in range(2):
                rbt = ps_sc.tile([P, 2 * NT], F32, tag="sc", name="rb")
                for u in range(2):
                    hl = 2 * pr + u
                    nc.tensor.matmul(
                        rbt[:, 0:NT],
                        sel_a if u == 0 else sel_b,
                        dens[:, hl * NT:(hl + 1) * NT],
                        start=(u == 0), stop=(u == 1))
                dsb = denp.tile([P, NT], F32, tag="dsb", name="dsb")
                nc.vector.tensor_copy(dsb, rbt[:, 0:NT])
                rcb = denp.tile([P, NT], F32, tag="rcb", name="rcb")
                nc.vector.reciprocal_approx_fast(out=rcb[:], in_=dsb[:])
                for u in range(2):
                    hl = 2 * pr + u
                    ct = ctxp.tile([D, NT], BF16, tag="ct", name="ct")
                    nc.vector.tensor_mul(ct, cxs[hl][0:D, :],
                                         rcb[u * D:(u + 1) * D, :])
                    nc.sync.dma_start(
                            out=a2a_in[qt][:, hl * D:(hl + 1) * D,
                                           :].rearrange("d p q -> p d q"),
                            in_=ct[:].rearrange("p (d q) -> p d q",
                                                d=N_CORES))
            nc.gpsimd.collective_compute(
                "AllToAll", ALU.bypass,
                replica_groups=[list(range(N_CORES))],
                ins=[a2a_in[qt][:].opt()], outs=[a2a_out[qt][:].opt()],
                unique_tensors="Yes")
            if qt >= 1:
                receiver(qt - 1)
        receiver(QT - 1)


def _prep_inputs(x, ln_g, ln_b, wqkv, bqkv, wo, bo):
    """Host-side sharding / folding. Returns per-core input dicts."""
    f32 = np.float32
    bf16 = ml_dtypes.bfloat16
    x = np.asarray(x, f32)
    wg = (np.asarray(wqkv, f32) * np.asarray(ln_g, f32)[:, None])
    tri = (np.arange(128)[None, :] >= np.arange(128)[:, None]).astype(bf16)
    wo_f = np.asarray(wo, f32)
    wo_bf = wo_f.astype(bf16)
    lnb = np.asarray(ln_b, f32)
    bq = np.asarray(bqkv, f32)
    bo_f = np.asarray(bo, f32)

    xT = [np.ascontiguousarray(x[b].T) for b in range(B)]
    xbf = [t.astype(bf16) for t in xT]

    # V bias folded through Wo: full ctx bias vector (all head groups)
    b2v_full = np.zeros(DIM, f32)
    for s in range(4):
        vs = slice(2 * DIM + DL * s, 2 * DIM + DL * s + DL)
        wv_f = wg[:, vs]
        b2v_full[DL * s:DL * s + DL] = bq[vs] + wv_f.T @ lnb
    bo2 = bo_f + b2v_full @ wo_f
    sel2 = np.zeros((2, 128), np.float32)
    sel2[0, 0:64] = 1.0
    sel2[1, 64:128] = 1.0
    sel2 = sel2.astype(bf16)

    maps = []
    qbias = False
    for c in range(N_CORES):
        b, s = divmod(c, 4)
        qs = slice(DL * s, DL * s + DL)
        ks = slice(DIM + DL * s, DIM + DL * s + DL)
        vs = slice(2 * DIM + DL * s, 2 * DIM + DL * s + DL)
        wqk_l = np.concatenate([wg[:, qs], wg[:, ks]], axis=1).astype(bf16)
        wv_l = wg[:, vs].astype(bf16)
        wqk_f = wqk_l.astype(f32)
        wv_f = wv_l.astype(f32)
        cqk = wqk_f.sum(0)                       # [512]
        b2q = bq[qs] + wqk_f[:, 0:DL].T @ lnb    # Q bias (post-scale ref!)
        cv = wv_f.sum(0)                         # [256]
        if np.abs(b2q).max() > 0:
            qbias = True
        augq = np.stack([-cqk, np.zeros(2 * DL, f32)]).astype(bf16)
        # tokens for core c: 512*qt + 64*c + i, cols ordered [qt][b2][64]
        toks = (512 * np.arange(QT)[:, None] + WC * c
                + np.arange(WC)[None, :]).reshape(-1)
        xres_c = np.stack([xT[b2][:, toks] for b2 in range(2)], axis=1)
        xres_c = xres_c.reshape(DIM, 2, QT, WC).transpose(0, 2, 1, 3)
        xres_c = np.ascontiguousarray(xres_c.reshape(DIM, 2 * WC * QT))
        maps.append({
            "xbf": xbf[b],
            "xres": xres_c,
            "wqk": wqk_l,
            "wv": wv_l,
            "wo": wo_bf,
            "augq": augq,
            "ncv": np.ascontiguousarray(-cv[None, :]).astype(bf16),
            "bq": np.ascontiguousarray(
                b2q.reshape(2, 128).T.astype(f32)),
            "tri": tri,
            "bo_col": np.ascontiguousarray(bo2.reshape(FT, 128).T),
            "sel": sel2,
        })
    maps_qbias = qbias
    return maps, maps_qbias


def kernel(**inputs):
    maps, qbias = _prep_inputs(**inputs)
    key = ("nc", qbias)
    if key not in _CACHE:
        _CACHE[key] = _build(qbias)
    _CACHE["nc"] = _CACHE[key]
    nc = _CACHE[key]
    res = run_bass_kernel_spmd(nc, maps, list(range(N_CORES)))
    out = np.empty((B, S, DIM), np.float32)
    for c in range(N_CORES):
        y = res.results[c]["y"]            # [DIM, 2*WC*QT]
        yv = y.reshape(DIM, QT, 2, WC)
        for b2 in range(2):
            for qt in range(QT):
                out[b2, 512 * qt + WC * c:512 * qt + WC * c + WC, :] = \
                    yv[:, qt, b2, :].T
    return out
